# revision 18
# baseline (speedup 1.0000x reference)
"""Trainium2 Bass kernel for AtlasTemporalMemoryAttnLayer.

Data-parallel over the 50000 destination rows across 8 NeuronCores (6272
padded rows / 49 tiles of 128 each per core).  Host prep: W_mem folded into
the Q/KV/out projections (memory rows feed matmuls directly), neighbor
memory rows pre-gathered, time encodings (cos) precomputed, biases folded
via an appended ones-row (Q/KV) or a rank-1 matmul (out proj), all
activations pre-transposed to feature-major bf16 and packed into per-tile
contiguous mega-blocks (k-major so the PE weight loads are contiguous and
get fast-weight-load).  On-chip: PE does all projections, attention runs
row-major split across Vector+GpSimd engines, LayerNorm via bn_stats.
"""

import numpy as np
import ml_dtypes

BF16 = ml_dtypes.bfloat16

NCORES = 8
TILE = 128
T = 49                      # tiles per core
R = TILE * T                # 6272 rows per core
NPAD = NCORES * R           # 50176
N_FULL = 50000
KNB = 16                    # neighbors
H, DH, DOUT, DN, DT = 2, 64, 128, 128, 100
N_MEM = 200000

_CACHE = {}


# ----------------------------------------------------------------------------
# device program
# ----------------------------------------------------------------------------
def _build_nc(n_tiles=T, rows=R):
    import concourse.bacc as bacc
    import concourse.tile as tile
    import concourse.bass as bass
    from concourse import mybir

    bf = mybir.dt.bfloat16
    f32 = mybir.dt.float32
    AF = mybir.ActivationFunctionType
    OP = mybir.AluOpType
    AX = mybir.AxisListType

    nc = bacc.Bacc("TRN2", target_bir_lowering=False, debug=False)

    # per-core inputs, pre-tiled on host:
    #  mega[t, f, 3*2048]: src | edge | gmem neighbor blocks, k-major cols
    #  tmeg[t, f(101), 2176]: tsrc (k-major 2048) | tdst (128)
    #  smal[t, f, 256]: dst (128) | gdst (128)
    mega = nc.declare_dram_parameter("mega", [n_tiles, 128, 6144], bf, isOutput=False)
    tmeg = nc.declare_dram_parameter("tmeg", [n_tiles, 101, 2176], bf, isOutput=False)
    smal = nc.declare_dram_parameter("smal", [n_tiles, 128, 256], bf, isOutput=False)
    # weights
    wqa = nc.declare_dram_parameter("wqa", [128, 128], bf, isOutput=False)
    wqb = nc.declare_dram_parameter("wqb", [128, 128], bf, isOutput=False)
    wqc = nc.declare_dram_parameter("wqc", [101, 128], bf, isOutput=False)
    kb1 = nc.declare_dram_parameter("kb1", [128, 256], bf, isOutput=False)
    kb2 = nc.declare_dram_parameter("kb2", [128, 256], bf, isOutput=False)
    kb3 = nc.declare_dram_parameter("kb3", [128, 256], bf, isOutput=False)
    kb4 = nc.declare_dram_parameter("kb4", [101, 256], bf, isOutput=False)
    c1 = nc.declare_dram_parameter("c1", [128, 128], bf, isOutput=False)
    c2 = nc.declare_dram_parameter("c2", [128, 128], bf, isOutput=False)
    c3 = nc.declare_dram_parameter("c3", [128, 128], bf, isOutput=False)
    boutr = nc.declare_dram_parameter("boutr", [1, 128], bf, isOutput=False)
    lng = nc.declare_dram_parameter("lng", [128, 128], f32, isOutput=False)
    lnb = nc.declare_dram_parameter("lnb", [128, 128], f32, isOutput=False)
    ident = nc.declare_dram_parameter("ident", [128, 128], bf, isOutput=False)
    out_d = nc.declare_dram_parameter("out", [rows, 128], f32, isOutput=True)

    with tile.TileContext(nc) as tc:
        with (
            tc.tile_pool(name="const", bufs=1) as const,
            tc.tile_pool(name="big", bufs=2) as big,
            tc.tile_pool(name="med", bufs=3) as med,
            tc.tile_pool(name="tiny", bufs=4) as tiny,
            tc.tile_pool(name="pkv", bufs=2, space="PSUM") as pkv,
            tc.tile_pool(name="ptp", bufs=2, space="PSUM") as ptp,
            tc.tile_pool(name="pqo", bufs=2, space="PSUM") as pqo,
        ):
            # resident constants
            wqa_s = const.tile([128, 128], bf); nc.sync.dma_start(wqa_s[:], wqa[:])
            wqb_s = const.tile([128, 128], bf); nc.sync.dma_start(wqb_s[:], wqb[:])
            wqc_s = const.tile([101, 128], bf); nc.sync.dma_start(wqc_s[:], wqc[:])
            kb1_s = const.tile([128, 256], bf); nc.sync.dma_start(kb1_s[:], kb1[:])
            kb2_s = const.tile([128, 256], bf); nc.sync.dma_start(kb2_s[:], kb2[:])
            kb3_s = const.tile([128, 256], bf); nc.sync.dma_start(kb3_s[:], kb3[:])
            kb4_s = const.tile([101, 256], bf); nc.sync.dma_start(kb4_s[:], kb4[:])
            c1_s = const.tile([128, 128], bf); nc.sync.dma_start(c1_s[:], c1[:])
            c2_s = const.tile([128, 128], bf); nc.sync.dma_start(c2_s[:], c2[:])
            c3_s = const.tile([128, 128], bf); nc.sync.dma_start(c3_s[:], c3[:])
            boutr_s = const.tile([1, 128], bf); nc.sync.dma_start(boutr_s[:], boutr[:])
            lng_s = const.tile([128, 128], f32); nc.sync.dma_start(lng_s[:], lng[:])
            lnb_s = const.tile([128, 128], f32); nc.sync.dma_start(lnb_s[:], lnb[:])
            ones_s = const.tile([1, 128], bf)
            nc.vector.memset(ones_s[:], 1.0)
            eps_s = const.tile([128, 1], f32)
            nc.vector.memset(eps_s[:], 1e-5)
            id_s = const.tile([128, 128], bf)
            nc.sync.dma_start(id_s[:], ident[:])

            # ---- 4-stage static software pipeline, 3 ticks deep ----
            def stage0(t):
                """loads + Q + KV matmuls + evictions (PE/ACT/sync)"""
                mg = big.tile([128, 6144], bf, tag="mg", bufs=3)
                nc.sync.dma_start(mg[:], mega[t])
                tg = big.tile([101, 2176], bf, tag="tg")
                nc.sync.dma_start(tg[:], tmeg[t])
                sm = med.tile([128, 256], bf, tag="sm", bufs=5)
                nc.sync.dma_start(sm[:], smal[t])

                srct = mg[:, 0:2048]
                edgt = mg[:, 2048:4096]
                gsrct = mg[:, 4096:6144]
                tsrct = tg[:, 0:2048]
                tdstt = tg[:, 2048:2176]
                dstt = sm[:, 0:128]
                gdstt = sm[:, 128:256]

                q_ps = pqo.tile([128, 128], f32, tag="q")
                nc.tensor.matmul(q_ps[:], dstt, wqa_s[:], start=True, stop=False)
                nc.tensor.matmul(q_ps[:], gdstt, wqb_s[:], start=False, stop=False)
                nc.tensor.matmul(q_ps[:], tdstt, wqc_s[:], start=False, stop=True)
                qsb = med.tile([128, 128], bf, tag="qsb", bufs=3)
                nc.scalar.copy(out=qsb[:], in_=q_ps[:])

                kvsb = big.tile([128, 4096], bf, tag="kvsb", bufs=4)
                for g in range(4):
                    kv_ps = pkv.tile([128, 1024], f32, tag="kv")
                    for j in range(4):
                        k = g * 4 + j
                        sl = kv_ps[:, j * 256:(j + 1) * 256]
                        ks = slice(k * 128, (k + 1) * 128)
                        nc.tensor.matmul(sl, srct[:, ks], kb1_s[:],
                                         start=True, stop=False)
                        nc.tensor.matmul(sl, gsrct[:, ks], kb2_s[:],
                                         start=False, stop=False)
                        nc.tensor.matmul(sl, edgt[:, ks], kb3_s[:],
                                         start=False, stop=False)
                        nc.tensor.matmul(sl, tsrct[:, ks], kb4_s[:],
                                         start=False, stop=True)
                    nc.scalar.copy(out=kvsb[:, g * 1024:(g + 1) * 1024],
                                   in_=kv_ps[:])
                return dict(kvsb=kvsb, qsb=qsb, dstt=dstt, gdstt=gdstt)

            def stage1(st):
                """QK scores + softmax (DVE + one ACT exp)"""
                kvsb, qsb = st["kvsb"], st["qsb"]
                kview = kvsb[:].rearrange("p (k c) -> p k c", c=256)[:, :, 0:128]
                qkp = big.tile([128, 2048], bf, tag="qkp")
                q_b = bass.AP(tensor=qsb.tensor, offset=qsb[:].offset,
                              ap=[qsb[:].ap[0], [0, KNB], [1, 128]])
                nc.vector.tensor_tensor(
                    out=qkp[:].rearrange("p (k c) -> p k c", c=128),
                    in0=kview, in1=q_b, op=OP.mult)
                qkh = med.tile([128, 1024], f32, tag="qkh")
                qkp_v = qkp[:].rearrange("p (kh d) -> p kh d", d=DH)
                nc.vector.tensor_tensor(
                    out=qkh[:].rearrange("p (kh d) -> p kh d", d=32),
                    in0=qkp_v[:, :, 0:32], in1=qkp_v[:, :, 32:64], op=OP.add)
                scores = tiny.tile([128, 32], f32, tag="scores")
                nc.vector.tensor_reduce(
                    out=scores[:],
                    in_=qkh[:].rearrange("p (kh d) -> p kh d", d=32),
                    axis=AX.X, op=OP.add)
                sc2 = tiny.tile([128, 32], f32, tag="sc2")
                nc.vector.scalar_tensor_tensor(out=sc2[:], in0=scores[:],
                                               scalar=0.2, in1=scores[:],
                                               op0=OP.mult, op1=OP.max)
                nmax = tiny.tile([128, 1], f32, tag="nmax")
                nc.vector.tensor_reduce(out=nmax[:], in_=sc2[:], axis=AX.X,
                                        op=OP.max, negate=True)
                e = tiny.tile([128, 32], bf, tag="e", bufs=6)
                nc.scalar.activation(out=e[:], in_=sc2[:], func=AF.Exp,
                                     bias=nmax[:, 0:1], scale=1.0)
                e_h = e[:].rearrange("p (k h) -> p h k", h=H)
                l = tiny.tile([128, 2], f32, tag="l")
                nc.vector.tensor_reduce(out=l[:], in_=e_h, axis=AX.X, op=OP.add)
                rl = tiny.tile([128, 2], f32, tag="rl", bufs=6)
                nc.vector.reciprocal(out=rl[:], in_=l[:])
                st["e"], st["rl"] = e, rl

            def stage2(st):
                """AV + attn transpose (DVE + PE)"""
                kvsb, e, rl = st["kvsb"], st["e"], st["rl"]
                avp = big.tile([128, 2048], bf, tag="avp")
                e_b = bass.AP(tensor=e.tensor, offset=e[:].offset,
                              ap=[e[:].ap[0], [2, KNB], [1, H], [0, DH]])
                v_b = kvsb[:].rearrange("p (k c) -> p k c", c=256)[
                    :, :, 128:256].rearrange("p k (h d) -> p k h d", h=H)
                nc.gpsimd.tensor_tensor(
                    out=avp[:].rearrange("p (k h d) -> p k h d", k=KNB, h=H),
                    in0=v_b, in1=e_b, op=OP.mult)
                avh = med.tile([128, 1024], f32, tag="avh")
                nc.gpsimd.tensor_tensor(out=avh[:], in0=avp[:, 0:1024],
                                        in1=avp[:, 1024:2048], op=OP.add)
                attn = med.tile([128, 128], f32, tag="attn")
                nc.vector.tensor_reduce(
                    out=attn[:],
                    in_=bass.AP(tensor=avh.tensor, offset=avh[:].offset,
                                ap=[avh[:].ap[0], [1, 128], [128, 8]]),
                    axis=AX.X, op=OP.add)
                attn_bf = med.tile([128, 128], bf, tag="attn_bf")
                for h in range(H):
                    nc.vector.tensor_scalar(out=attn_bf[:, h * DH:(h + 1) * DH],
                                            in0=attn[:, h * DH:(h + 1) * DH],
                                            scalar1=rl[:, h:h + 1], scalar2=None,
                                            op0=OP.mult)
                tpa = ptp.tile([128, 128], bf, tag="o2")
                nc.tensor.transpose(out=tpa[:], in_=attn_bf[:], identity=id_s[:])
                attnT = med.tile([128, 128], bf, tag="attnT", bufs=3)
                nc.vector.tensor_scalar(out=attnT[:], in0=tpa[:], scalar1=1.0,
                                        scalar2=None, op0=OP.mult)
                st["attnT"] = attnT

            def stage3(t, st):
                """out projection + relu + layernorm + store (PE/DVE/ACT)"""
                rb = t * TILE
                attnT, dstt, gdstt = st["attnT"], st["dstt"], st["gdstt"]
                o2_ps = ptp.tile([128, 128], f32, tag="o2")
                nc.tensor.matmul(o2_ps[:], attnT[:], c1_s[:], start=True, stop=False)
                nc.tensor.matmul(o2_ps[:], dstt, c2_s[:], start=False, stop=False)
                nc.tensor.matmul(o2_ps[:], gdstt, c3_s[:], start=False, stop=False)
                nc.tensor.matmul(o2_ps[:], ones_s[:], boutr_s[:], start=False,
                                 stop=True)
                o2r = med.tile([128, 128], f32, tag="o2r")
                nc.vector.tensor_scalar(out=o2r[:], in0=o2_ps[:], scalar1=0.0,
                                        scalar2=None, op0=OP.max)
                stats = tiny.tile([128, 6], f32, tag="stats")
                nc.vector.bn_stats(out=stats[:], in_=o2r[:])
                mv = tiny.tile([128, 2], f32, tag="mv")
                nc.vector.bn_aggr(out=mv[:], in_=stats[:])
                sd = tiny.tile([128, 1], f32, tag="sd")
                nc.scalar.activation(out=sd[:], in_=mv[:, 1:2], func=AF.Sqrt,
                                     bias=eps_s[:], scale=1.0)
                rs = tiny.tile([128, 1], f32, tag="rs")
                nc.vector.reciprocal(out=rs[:], in_=sd[:])
                t1 = med.tile([128, 128], f32, tag="t1")
                nc.vector.scalar_tensor_tensor(out=t1[:], in0=o2r[:],
                                               scalar=mv[:, 0:1], in1=lng_s[:],
                                               op0=OP.subtract, op1=OP.mult)
                outsb = med.tile([128, 128], f32, tag="outsb")
                nc.vector.scalar_tensor_tensor(out=outsb[:], in0=t1[:],
                                               scalar=rs[:, 0:1], in1=lnb_s[:],
                                               op0=OP.mult, op1=OP.add)
                nc.sync.dma_start(out=out_d[rb:rb + TILE, :], in_=outsb[:])

            # emission order per tick: S3(t-3), S2(t-2), S0(t), S1(t-1) —
            # ScalarE sees the next tile's evictions before the exp that
            # depends on this tick's DVE chain.
            states = {}
            for tick in range(n_tiles + 3):
                if tick >= 3:
                    stage3(tick - 3, states.pop(tick - 3))
                if tick >= 2 and tick - 2 < n_tiles:
                    stage2(states[tick - 2])
                if tick < n_tiles:
                    states[tick] = stage0(tick)
                if tick >= 1 and tick - 1 < n_tiles:
                    stage1(states[tick - 1])

    nc.compile()
    return nc


# ----------------------------------------------------------------------------
# host side
# ----------------------------------------------------------------------------
def _host_prep(inputs, rows=R, n_tiles=T):
    """Returns list of 8 per-core input dicts."""
    f32 = np.float32

    def a(x, dt=f32):
        return np.asarray(x, dtype=dt)

    memory = a(inputs["memory"])
    dst_feat = a(inputs["dst_feat"])
    src_feat = a(inputs["src_feat"])
    edge_feat = a(inputs["edge_feat"])
    dst_ts = a(inputs["dst_ts"])
    src_ts = a(inputs["src_ts"])
    dst_nodes = np.asarray(inputs["dst_nodes"]).astype(np.int64)
    src_nodes = np.asarray(inputs["src_nodes"]).astype(np.int64)
    W_mem = a(inputs["W_mem"]); b_mem = a(inputs["b_mem"])
    time_w = a(inputs["time_w"]); time_b = a(inputs["time_b"])
    W_q = a(inputs["W_q"]); b_q = a(inputs["b_q"])
    W_kv = a(inputs["W_kv"]); b_kv = a(inputs["b_kv"])
    W_out = a(inputs["W_out"]); b_out = a(inputs["b_out"])
    ln_g = a(inputs["ln_g"]); ln_b = a(inputs["ln_b"])

    n = dst_feat.shape[0]
    npad = NCORES * rows
    pad = npad - n

    def padrows(x):
        if pad == 0:
            return x
        return np.concatenate([x, np.zeros((pad,) + x.shape[1:], x.dtype)], axis=0)

    dst_feat = padrows(dst_feat); src_feat = padrows(src_feat)
    edge_feat = padrows(edge_feat)
    dst_ts = padrows(dst_ts); src_ts = padrows(src_ts)
    dst_nodes = padrows(dst_nodes); src_nodes = padrows(src_nodes)

    Wq1, Wq3 = W_q[:, :DN], W_q[:, DN:DN + DT]
    Wkv1, Wkv2, Wkv3 = W_kv[:, :DN], W_kv[:, DN:2 * DN], W_kv[:, 2 * DN:]
    Wout1, Wout2 = W_out[:, :DOUT], W_out[:, DOUT:]
    bq_eff = b_q + Wq1 @ b_mem
    bkv_eff = b_kv + Wkv1 @ b_mem
    bout_eff = b_out + Wout2 @ b_mem

    bfc = lambda x: np.ascontiguousarray(x, dtype=BF16)
    mem_bf = memory.astype(BF16)
    shared = {
        "wqa": bfc(Wq1.T), "wqb": bfc((Wq1 @ W_mem).T),
        "wqc": bfc(np.concatenate([Wq3.T, bq_eff[None, :]], axis=0)),
        "kb1": bfc(Wkv1.T), "kb2": bfc((Wkv1 @ W_mem).T), "kb3": bfc(Wkv2.T),
        "kb4": bfc(np.concatenate([Wkv3.T, bkv_eff[None, :]], axis=0)),
        "c1": bfc(Wout1.T), "c2": bfc(Wout2.T), "c3": bfc((Wout2 @ W_mem).T),
        "boutr": bfc(bout_eff[None, :]),
        "ident": bfc(np.eye(128, dtype=f32)),
        "lng": np.ascontiguousarray(np.broadcast_to(ln_g[None, :], (128, 128)), f32),
        "lnb": np.ascontiguousarray(np.broadcast_to(ln_b[None, :], (128, 128)), f32),
    }

    def kmaj(x, nt=n_tiles):
        # [rows, KNB, F] -> [nt, F, KNB*TILE] (k-major per tile)
        f = x.shape[-1]
        return np.ascontiguousarray(
            x.reshape(nt, TILE, KNB, f).transpose(0, 3, 2, 1).reshape(
                nt, f, KNB * TILE))

    def ftile(x, nt=n_tiles):
        # [rows, F] -> [nt, F, TILE]
        return np.ascontiguousarray(
            x.reshape(nt, TILE, -1).transpose(0, 2, 1))

    in_maps = []
    for c in range(NCORES):
        s = slice(c * rows, (c + 1) * rows)
        dts = dst_ts[s]; sts = src_ts[s]
        delta = np.maximum(dts[:, None] - sts, 0.0)
        tsrc = np.cos(delta[..., None] * time_w + time_b).astype(BF16)
        tdst = np.cos(dts[:, None] * time_w + time_b).astype(BF16)

        mega = np.concatenate([
            kmaj(src_feat[s].astype(BF16)),
            kmaj(edge_feat[s].astype(BF16)),
            kmaj(mem_bf[src_nodes[s]]),
        ], axis=2)                                      # [T,128,6144]
        ones_rk = np.ones((n_tiles, 1, KNB * TILE), BF16)
        ones_r = np.ones((n_tiles, 1, TILE), BF16)
        tmeg = np.concatenate([
            np.concatenate([kmaj(tsrc), ones_rk], axis=1),
            np.concatenate([ftile(tdst), ones_r], axis=1),
        ], axis=2)                                      # [T,101,2176]
        smal = np.concatenate([
            ftile(dst_feat[s].astype(BF16)),
            ftile(mem_bf[dst_nodes[s]]),
        ], axis=2)                                      # [T,128,256]
        m = {"mega": np.ascontiguousarray(mega),
             "tmeg": np.ascontiguousarray(tmeg),
             "smal": np.ascontiguousarray(smal)}
        m.update(shared)
        in_maps.append(m)
    return in_maps


LAST_RESULTS = None


def kernel(**inputs):
    global LAST_RESULTS
    from concourse.bass_utils import run_bass_kernel_spmd
    import os

    if "nc" not in _CACHE:
        _CACHE["nc"] = _build_nc()
    nc = _CACHE["nc"]

    in_maps = _host_prep(inputs)
    trace = bool(os.environ.get("BASS_TRACE"))
    if trace:
        try:
            from antenv.axon_hooks import set_axon_ntff_profile_hook
            from trn_agent_boot.trn_boot import _ntff_profile_via_ctypes
            set_axon_ntff_profile_hook(
                _ntff_profile_via_ctypes("/opt/axon/libaxon_pjrt.so"))
        except Exception:
            pass
    res = run_bass_kernel_spmd(nc, in_maps, core_ids=list(range(NCORES)),
                               trace=trace)
    LAST_RESULTS = res
    out = np.concatenate([np.asarray(res.results[c]["out"])
                          for c in range(NCORES)], axis=0)
    return out[:N_FULL].astype(np.float32)
